# revision 1
# baseline (speedup 1.0000x reference)
"""FastSelfAttention Trainium2 kernel (batched two-phase, bf16 I/O).

Reference computation (B=4, S=4096, D=1024):
    h  = layer_norm(hidden_states, g, b)
    q  = h @ Wq.T ; k = h @ Wk.T ; v = q
    qw = exp((q @ wq_att) / sqrt(D) + mask)
    pq = cumsum(qw * q, S) / cumsum(qw, S)
    mk = pq * k
    kw = exp((mk @ wk_att) / sqrt(D) + mask)
    pk = cumsum(kw * mk, S) / cumsum(kw, S)
    out = pk * v

Sharding: 8 cores = 4 batches x 2 halves of the feature (e) dimension.
Layout on device is feature-major [e, s]; cumsum runs along the free
(s) axis via DVE tensor_tensor_scan, chained across s-chunks with
carry columns.  The second pooling's logit l2 needs the full e range:
sweep-1 runs over a 4-chunk super-batch, one pairwise AllReduce
([[0,1],[2,3],[4,5],[6,7]]) combines the halves, then pool-2 replays
those chunks while the next super-batch's sweep-1 overlaps the AR.

LayerNorm folding: with xs[d,s] = h[d,s]*rstd[s] the projection is
    q[e,s] = sum_d W'q[e,d] xs[d,s] + nmur[s] colsq[e] + cq[e]
(nmur = -mu*rstd); the two rank-1 corrections ride one K=2 matmul
(stationary [colsq; cq], moving rows [nmur; ones]).
l1 = rstd * ((vqp - colsvq/D).h) + (b.vq/sqrt(D) + mask) so the mean
subtraction is folded into the l1 stationary host-side.

Everything streams in bf16 (h input, weights, q/k/mk, output); scans
and denominators accumulate in f32.
"""

import numpy as np
import ml_dtypes

import concourse.bass as bass
import concourse.bacc as bacc
import concourse.mybir as mybir
import concourse.tile as tile
from concourse.bass_utils import run_bass_kernel_spmd

dt = mybir.dt
AF = mybir.ActivationFunctionType
OP = mybir.AluOpType

B, S, D = 4, 4096, 1024
EH = D // 2          # e-half per core
NC = 8               # cores
SC = 512             # s-chunk
NSC = S // SC        # 8 s-chunks
NB = 2               # AllReduce super-batches
CPB = NSC // NB      # chunks per super-batch (4)
SB = SC * CPB        # tokens per super-batch (2048)
ND = D // 128        # 8 d-chunks
NE = EH // 128       # 4 e-chunks per core
INV_SQRT_D = 1.0 / np.sqrt(np.float32(D))
EPS = 1e-5

_prog_cache = {}


def _build_program(no_collective=False):
    key = ("ncb", no_collective)
    if key in _prog_cache:
        return _prog_cache[key]

    nc = bacc.Bacc("TRN2", num_devices=NC)
    f32, bf16 = dt.float32, dt.bfloat16

    # ---- external I/O (all big tensors bf16) ----
    hb = nc.dram_tensor("hb", [D, S], bf16, kind="ExternalInput")
    wqT = nc.dram_tensor("wqT", [D, EH], bf16, kind="ExternalInput")
    wkT = nc.dram_tensor("wkT", [D, EH], bf16, kind="ExternalInput")
    svq_in = nc.dram_tensor("svq", [ND, 128], bf16, kind="ExternalInput")
    ccq_in = nc.dram_tensor("ccq", [2, EH], bf16, kind="ExternalInput")
    cck_in = nc.dram_tensor("cck", [2, EH], bf16, kind="ExternalInput")
    wkp_in = nc.dram_tensor("wkp", [NE, 128], bf16, kind="ExternalInput")
    mrow1_in = nc.dram_tensor("mrow1", [1, S], f32, kind="ExternalInput")
    mrow2_in = nc.dram_tensor("mrow2", [1, S], f32, kind="ExternalInput")
    r2init_in = nc.dram_tensor("r2init", [2, SC], bf16, kind="ExternalInput")

    outT = nc.dram_tensor("outT", [EH, S], bf16, kind="ExternalOutput")

    with tile.TileContext(nc) as tc:
        with (
            tc.tile_pool(name="const", bufs=1) as cpool,
            tc.tile_pool(name="persist", bufs=1) as ppool,
            tc.tile_pool(name="rows", bufs=1) as rows,
            tc.tile_pool(name="work", bufs=2) as wk,
            tc.tile_pool(name="work1", bufs=1) as wk1,
            tc.tile_pool(name="work3", bufs=3) as wk3,
            tc.tile_pool(name="psA", bufs=2, space="PSUM") as psA,
            tc.tile_pool(name="psB", bufs=2, space="PSUM") as psB,
            tc.tile_pool(name="psR", bufs=1, space="PSUM") as psR,
            tc.tile_pool(name="psL2", bufs=1, space="PSUM") as psL2,
            tc.tile_pool(name="dram", bufs=1, space="DRAM") as dpool,
        ):
            # ---- resident constants (one DMA each) ----
            wq_t = cpool.tile([128, ND, EH], bf16, tag="wq")
            wk_t = cpool.tile([128, ND, EH], bf16, tag="wk")
            nc.sync.dma_start(
                out=wq_t[:], in_=wqT.rearrange("(a p) e -> p a e", p=128))
            nc.sync.dma_start(
                out=wk_t[:], in_=wkT.rearrange("(a p) e -> p a e", p=128))

            svq_t = cpool.tile([128, ND], bf16, tag="svq")
            nc.sync.dma_start(out=svq_t[:], in_=svq_in.transpose([1, 0]))

            ccq_t = cpool.tile([2, EH], bf16, tag="ccq")
            cck_t = cpool.tile([2, EH], bf16, tag="cck")
            nc.sync.dma_start(out=ccq_t[:], in_=ccq_in[:])
            nc.sync.dma_start(out=cck_t[:], in_=cck_in[:])

            wkp_t = cpool.tile([128, NE], bf16, tag="wkp")
            nc.sync.dma_start(out=wkp_t[:], in_=wkp_in.transpose([1, 0]))

            r2 = cpool.tile([2, SC], bf16, tag="r2")
            nc.sync.dma_start(out=r2[:], in_=r2init_in[:])

            ones_rk1 = cpool.tile([1, 128], bf16, tag="ones_rk1")
            nc.vector.memset(ones_rk1[:], 1.0)
            ones1 = cpool.tile([128, 1], bf16, tag="ones1")
            nc.vector.memset(ones1[:], 1.0)
            ones8 = cpool.tile([128, 2, 16], dt.float8e4, tag="ones8")
            nc.vector.memset(ones8[:], 1.0)
            eps_t = cpool.tile([1, 1], f32, tag="eps")
            nc.vector.memset(eps_t[:], EPS)

            # ---- persistent state ----
            carry_q = ppool.tile([128, NE], f32, tag="carry_q")
            carry_k = ppool.tile([128, NE], f32, tag="carry_k")
            carry_d = ppool.tile([1, 2], f32, tag="carry_d")
            nc.vector.memset(carry_q[:], 0.0)
            nc.vector.memset(carry_k[:], 0.0)
            nc.vector.memset(carry_d[:], 0.0)

            l2p_dram = dpool.tile([1, S], bf16, tag="l2p")
            l2f_dram = dpool.tile([1, S], bf16, tag="l2f")

            for g in range(NB):
                g0 = g * SB
                # super-batch state (double-buffered across batches)
                q_t = {}
                mk_t = {}
                l2acc = rows.tile([1, SB], bf16, tag="l2acc")

                # ================= sweep 1 =================
                for cc in range(CPB):
                    s0 = g0 + cc * SC

                    ht_c = wk3.tile([128, ND, SC], bf16, tag="ht")
                    nc.sync.dma_start(
                        out=ht_c[:],
                        in_=hb.rearrange("(a p) s -> p a s",
                                         p=128)[:, :, s0:s0 + SC])

                    sq_t = wk1.tile([128, ND, SC], dt.float8e4, tag="sq")
                    nc.scalar.activation(sq_t[:], ht_c[:], AF.Square)
                    h8_t = wk1.tile([128, ND, SC], dt.float8e4, tag="h8")
                    nc.vector.tensor_copy(h8_t[:], ht_c[:])

                    # stats + l1 rows
                    st_ps = psR.tile([1, SC], f32, tag="st")
                    for d in range(0, ND, 2):
                        nc.tensor.matmul(
                            st_ps[:], ones8[:, :, 0:1], h8_t[:, d:d + 2, :],
                            start=(d == 0), stop=(d == ND - 2),
                            perf_mode=mybir.MatmulPerfMode.DoubleRow)
                    l1_ps = psR.tile([1, SC], f32, tag="l1")
                    for d in range(ND):
                        nc.tensor.matmul(l1_ps[:], svq_t[:, d:d + 1],
                                         ht_c[:, d, :],
                                         start=(d == 0), stop=(d == ND - 1))
                    sxx_ps = psR.tile([1, SC], f32, tag="sxx")
                    for d in range(0, ND, 2):
                        nc.tensor.matmul(
                            sxx_ps[:], ones8[:, :, 0:1], sq_t[:, d:d + 2, :],
                            start=(d == 0), stop=(d == ND - 2),
                            perf_mode=mybir.MatmulPerfMode.DoubleRow)

                    # LN rows
                    negmu = rows.tile([1, SC], f32, tag="negmu")
                    nc.vector.tensor_scalar_mul(negmu[:], st_ps[:], -1.0 / D)
                    musq = rows.tile([1, SC], f32, tag="scratch")
                    nc.scalar.activation(musq[:], st_ps[:], AF.Square,
                                         scale=1.0 / D)
                    var = rows.tile([1, SC], f32, tag="var")
                    nc.vector.scalar_tensor_tensor(
                        var[:], sxx_ps[:], 1.0 / D, musq[:],
                        OP.mult, OP.subtract)
                    sd = rows.tile([1, SC], f32, tag="scratch")
                    nc.scalar.activation(sd[:], var[:], AF.Sqrt, bias=eps_t[:])
                    rstd = rows.tile([1, SC], f32, tag="rstd")
                    rscr = rows.tile([1, SC], f32, tag="rscr")
                    nc.vector.reciprocal_approx_accurate(rstd[:], sd[:],
                                                         rscr[:])
                    rstd_h = rows.tile([1, SC], bf16, tag="rstd_h")
                    nc.vector.tensor_copy(rstd_h[:], rstd[:])

                    # rank-1 moving rows [nmur; ones] (row1 DMA-initialized)
                    nc.vector.tensor_mul(r2[0:1, :], negmu[:], rstd[:])

                    # rstd broadcast -> xs
                    rb_ps = psB.tile([128, SC], f32, tag="bcast")
                    nc.tensor.matmul(rb_ps[:], ones_rk1[:], rstd_h[:],
                                     start=True, stop=True)
                    rb_sb = wk1.tile([128, SC], bf16, tag="rb_sb")
                    nc.scalar.copy(rb_sb[:], rb_ps[:])
                    hh = ND // 2
                    for p in range(2):
                        nc.vector.tensor_mul(
                            ht_c[:, p * hh:(p + 1) * hh, :],
                            ht_c[:, p * hh:(p + 1) * hh, :],
                            rb_sb[:].unsqueeze(1).broadcast_to([128, hh, SC]))
                    xs_t = ht_c

                    # l1 -> qw
                    l1f = rows.tile([1, SC], f32, tag="l1f")
                    nc.vector.tensor_mul(l1f[:], l1_ps[:], rstd[:])
                    m1s = rows.tile([1, SC], f32, tag="m1s")
                    nc.sync.dma_start(out=m1s[:], in_=mrow1_in[:, s0:s0 + SC])
                    l1b = rows.tile([1, SC], f32, tag="l1b")
                    nc.vector.tensor_add(l1b[:], l1f[:], m1s[:])
                    qw = rows.tile([1, SC], bf16, tag="qw")
                    nc.scalar.activation(qw[:], l1b[:], AF.Exp)

                    qb_sb = wk1.tile([128, SC], bf16, tag="qb_sb")
                    nc.gpsimd.partition_broadcast(qb_sb[:], qw[:])

                    # den1 scan + reciprocal
                    den1 = rows.tile([1, SC], f32, tag="den1")
                    init1 = 0.0 if s0 == 0 else carry_d[:, 0:1]
                    nc.vector.tensor_tensor_scan(
                        den1[:], qw[:], qw[:], init1, OP.add, OP.bypass)
                    nc.vector.tensor_copy(carry_d[:, 0:1], den1[:, SC - 1:SC])
                    rden1 = rows.tile([1, SC], f32, tag="rden1")
                    nc.vector.reciprocal_approx_accurate(
                        rden1[:], den1[:], rscr[:])
                    rden1h = rows.tile([1, SC], bf16, tag="rden1h")
                    nc.vector.tensor_copy(rden1h[:], rden1[:])

                    # phase A: projections + pool1 scans
                    n1_t = [None] * NE
                    for e in range(NE):
                        es = slice(e * 128, (e + 1) * 128)
                        q_ps = psA.tile([128, SC], f32, tag="proj")
                        for d in range(ND):
                            nc.tensor.matmul(
                                q_ps[:], wq_t[:, d, es], xs_t[:, d, :],
                                start=(d == 0), stop=False)
                        nc.tensor.matmul(q_ps[:], ccq_t[:, es], r2[:],
                                         start=False, stop=True)
                        qt = wk.tile([128, SC], bf16, tag=f"q{e}_{cc}")
                        nc.scalar.copy(qt[:], q_ps[:])
                        q_t[(e, cc)] = qt

                        k_ps = psA.tile([128, SC], f32, tag="proj")
                        for d in range(ND):
                            nc.tensor.matmul(
                                k_ps[:], wk_t[:, d, es], xs_t[:, d, :],
                                start=(d == 0), stop=False)
                        nc.tensor.matmul(k_ps[:], cck_t[:, es], r2[:],
                                         start=False, stop=True)
                        kt = wk1.tile([128, SC], bf16, tag=f"k{e}")
                        nc.scalar.copy(kt[:], k_ps[:])

                        u1 = wk1.tile([128, SC], bf16, tag="u1")
                        nc.vector.tensor_mul(u1[:], qb_sb[:], qt[:])
                        n1 = wk1.tile([128, SC], f32, tag=f"n1{e}")
                        initq = 0.0 if s0 == 0 else carry_q[:, e:e + 1]
                        nc.vector.tensor_tensor_scan(
                            n1[:], u1[:], u1[:], initq, OP.add, OP.bypass)
                        nc.vector.tensor_copy(carry_q[:, e:e + 1],
                                              n1[:, SC - 1:SC])
                        n1_t[e] = (n1, kt)

                    db_sb = wk1.tile([128, SC], bf16, tag="db_sb")
                    nc.gpsimd.partition_broadcast(db_sb[:], rden1h[:])

                    # phase B: mk + l2 partial
                    l2_ps = psL2.tile([1, SC], f32, tag="l2")
                    for e in range(NE):
                        n1, kt = n1_t[e]
                        pq = wk1.tile([128, SC], bf16, tag="pq")
                        nc.vector.tensor_mul(pq[:], n1[:], db_sb[:])
                        mk = wk.tile([128, SC], bf16, tag=f"mk{e}_{cc}")
                        nc.vector.tensor_mul(mk[:], pq[:], kt[:])
                        mk_t[(e, cc)] = mk
                        nc.tensor.matmul(l2_ps[:], wkp_t[:, e:e + 1], mk[:],
                                         start=(e == 0), stop=(e == NE - 1))
                    nc.vector.tensor_copy(
                        l2acc[:, cc * SC:(cc + 1) * SC], l2_ps[:])

                # ===== AllReduce: g0 whole; g1 split in halves so the
                # ===== first half of the final pool-2 overlaps sweep-1
                HB2 = SB // 2
                halves = [(0, SB)] if g == 0 else [(0, HB2), (HB2, SB)]
                for (h0, h1) in halves:
                    nc.sync.dma_start(out=l2p_dram[:, g0 + h0:g0 + h1],
                                      in_=l2acc[:, h0:h1])
                    if no_collective:
                        nc.sync.dma_start(out=l2f_dram[:, g0 + h0:g0 + h1],
                                          in_=l2p_dram[:, g0 + h0:g0 + h1])
                    else:
                        nc.gpsimd.collective_compute(
                            "AllReduce", OP.add,
                            replica_groups=[[0, 1], [2, 3], [4, 5], [6, 7]],
                            ins=[l2p_dram[:, g0 + h0:g0 + h1]],
                            outs=[l2f_dram[:, g0 + h0:g0 + h1]],
                        )
                l2s = rows.tile([1, SB], bf16, tag="l2s")
                for (h0, h1) in halves:
                    nc.sync.dma_start(out=l2s[:, h0:h1],
                                      in_=l2f_dram[:, g0 + h0:g0 + h1])
                m2s = rows.tile([1, SB], f32, tag="m2sg")
                nc.sync.dma_start(out=m2s[:], in_=mrow2_in[:, g0:g0 + SB])

                # ================= pool 2 =================
                for cc in range(CPB):
                    s0 = g0 + cc * SC
                    sl = slice(cc * SC, (cc + 1) * SC)
                    lg2 = rows.tile([1, SC], f32, tag="lg2")
                    nc.vector.tensor_add(lg2[:], l2s[:, sl], m2s[:, sl])
                    kw = rows.tile([1, SC], bf16, tag="kw")
                    nc.scalar.activation(kw[:], lg2[:], AF.Exp)
                    kb_ps = psB.tile([128, SC], f32, tag="bcast")
                    nc.tensor.matmul(kb_ps[:], ones_rk1[:], kw[:],
                                     start=True, stop=True)
                    kb_sb = wk1.tile([128, SC], bf16, tag="kb_sb")
                    nc.scalar.copy(kb_sb[:], kb_ps[:])

                    den2 = rows.tile([1, SC], f32, tag="den2")
                    init2 = 0.0 if s0 == 0 else carry_d[:, 1:2]
                    nc.vector.tensor_tensor_scan(
                        den2[:], kw[:], kw[:], init2, OP.add, OP.bypass)
                    nc.vector.tensor_copy(carry_d[:, 1:2], den2[:, SC - 1:SC])
                    rden2 = rows.tile([1, SC], f32, tag="rden2")
                    rscr2 = rows.tile([1, SC], f32, tag="rscr2")
                    nc.vector.reciprocal_approx_accurate(
                        rden2[:], den2[:], rscr2[:])
                    rden2h = rows.tile([1, SC], bf16, tag="rden2h")
                    nc.vector.tensor_copy(rden2h[:], rden2[:])
                    d2_ps = psB.tile([128, SC], f32, tag="bcast")
                    nc.tensor.matmul(d2_ps[:], ones_rk1[:], rden2h[:],
                                     start=True, stop=True)
                    d2_sb = wk1.tile([128, SC], bf16, tag="d2_sb")
                    nc.scalar.copy(d2_sb[:], d2_ps[:])

                    o_t = wk1.tile([128, NE, SC], bf16, tag="o")
                    for e in range(NE):
                        u2 = wk1.tile([128, SC], bf16, tag="u2")
                        nc.vector.tensor_mul(u2[:], kb_sb[:],
                                             mk_t[(e, cc)][:])
                        n2 = wk1.tile([128, SC], f32, tag="n2")
                        initk = 0.0 if s0 == 0 else carry_k[:, e:e + 1]
                        nc.vector.tensor_tensor_scan(
                            n2[:], u2[:], u2[:], initk, OP.add, OP.bypass)
                        nc.vector.tensor_copy(carry_k[:, e:e + 1],
                                              n2[:, SC - 1:SC])
                        pk = wk1.tile([128, SC], bf16, tag="pk")
                        nc.vector.tensor_mul(pk[:], n2[:], d2_sb[:])
                        nc.vector.tensor_mul(o_t[:, e, :], pk[:],
                                             q_t[(e, cc)][:])
                    nc.sync.dma_start(
                        out=outT.rearrange("(a p) s -> p a s",
                                           p=128)[:, :, s0:s0 + SC],
                        in_=o_t[:])

    nc.finalize()
    _prog_cache[key] = nc
    return nc


def _host_prep(hidden_states, attention_mask, Wq, wq_att, Wk, wk_att, ln_g, ln_b):
    """Build the 8 per-core input maps."""
    f4 = np.float32
    g = np.asarray(ln_g, f4)
    bb = np.asarray(ln_b, f4)
    Wq = np.asarray(Wq, f4)
    Wk = np.asarray(Wk, f4)
    wq_att = np.asarray(wq_att, f4)[:, 0]
    wk_att = np.asarray(wk_att, f4)[:, 0]
    h = np.asarray(hidden_states, f4)
    am = np.asarray(attention_mask, f4)

    Wqp = Wq * g[None, :]           # [e,d]
    Wkp = Wk * g[None, :]
    wqT_full = np.ascontiguousarray(Wqp.T)   # [d,e]
    wkT_full = np.ascontiguousarray(Wkp.T)
    cq_full = Wq @ bb               # [e]
    ck_full = Wk @ bb
    colsq_full = Wqp.sum(axis=1)    # [e]
    colsk_full = Wkp.sum(axis=1)

    vq = Wq.T @ wq_att              # [d]
    vqp = (g * vq) * INV_SQRT_D     # [d]
    cvq = float(bb @ vq) * INV_SQRT_D
    colsvq = float(vqp.sum())
    wkp_full = (wk_att * INV_SQRT_D).astype(f4)

    maskb = (1.0 - am) * -10000.0   # [B,S]

    def bf(a):
        return np.ascontiguousarray(np.asarray(a, f4).astype(ml_dtypes.bfloat16))

    # l1 stationary with mean-subtraction folded: vqp - colsvq/D
    svq = (vqp - colsvq / D).reshape(ND, 128)

    in_maps = []
    for core in range(NC):
        b, half = divmod(core, 2)
        sl = slice(half * EH, (half + 1) * EH)
        ccq = np.stack([colsq_full[sl], cq_full[sl]], axis=0)   # [2, EH]
        cck = np.stack([colsk_full[sl], ck_full[sl]], axis=0)
        in_maps.append({
            "hb": bf(h[b].T),
            "wqT": bf(wqT_full[:, sl]),
            "wkT": bf(wkT_full[:, sl]),
            "svq": bf(svq),
            "ccq": bf(ccq),
            "cck": bf(cck),
            "wkp": bf(wkp_full[sl].reshape(NE, 128)),
            "mrow1": np.ascontiguousarray((maskb[b] + cvq).reshape(1, S)),
            "mrow2": np.ascontiguousarray(maskb[b].reshape(1, S)),
            "r2init": bf(np.vstack([np.zeros(SC, f4), np.ones(SC, f4)])),
        })
    return in_maps


def kernel(**inputs):
    import time as _time
    nc = _build_program()
    in_maps = _host_prep(**inputs)
    res = None
    last = None
    for _attempt in range(3):
        try:
            res = run_bass_kernel_spmd(nc, in_maps, core_ids=list(range(NC)))
            break
        except Exception as e:  # transient first-exec device faults self-heal
            last = e
            _time.sleep(3)
    if res is None:
        raise last
    out = np.empty((B, S, D), np.float32)
    for core in range(NC):
        b, half = divmod(core, 2)
        out[b, :, half * EH:(half + 1) * EH] = \
            res.results[core]["outT"].astype(np.float32).T
    return out



# revision 22
# speedup vs baseline: 1.3798x; 1.3798x over previous
"""FastSelfAttention Trainium2 kernel — zero-collective batch-per-core.

Reference computation (B=4, S=4096, D=1024):
    h  = layer_norm(hidden_states, g, b)
    q  = h @ Wq.T ; k = h @ Wk.T ; v = q
    qw = exp((q @ wq_att) / sqrt(D) + mask)
    pq = cumsum(qw * q, S) / cumsum(qw, S)
    mk = pq * k
    kw = exp((mk @ wk_att) / sqrt(D) + mask)
    pk = cumsum(kw * mk, S) / cumsum(kw, S)
    out = pk * v

Sharding: one FULL batch per core (cores 4-7 duplicate batches 0-3), so
there are NO collectives and no cross-core sync — each core's NEFF runs
its own batch start-to-finish.  Layout is feature-major [e, s]; cumsum
runs along the free (s) axis via DVE tensor_tensor_scan, chained across
s-chunks with carry columns.

LayerNorm folding: xs = (h - mu) * rstd computed in-place from two
partition-broadcast rows (rstd, -mu*rstd); weights are pre-scaled by g
host-side (W' = W * g), and the bias contribution (W @ b, zero for this
problem's ln_b) rides the PSUM->SBUF copy as a per-partition bias.
l1 = rstd * ((vqp - colsvq/D).h) + (b.vq/sqrt(D) + mask) folds the mean
subtraction into the l1 stationary host-side; the token sum (for mu)
rides the same stationary as a second column.

Per-chunk phases are software-pipelined one chunk apart (sweep(c) ||
pool2(c-1)) so the PE never stalls on the DVE scan chain.  Engine map:
PE stats+projections+l2, DVE scans+big elementwise, Act psum copies +
exp/ln rows, Pool partition-broadcasts + final out-mul + carry copies.
All activation functions (Copy/Square/Exp/Ln) live in one table set.
"""

import numpy as np
import ml_dtypes

import concourse.bass as bass
import concourse.bacc as bacc
import concourse.mybir as mybir
import concourse.tile as tile
from concourse.bass_utils import run_bass_kernel_spmd

dt = mybir.dt
AF = mybir.ActivationFunctionType
OP = mybir.AluOpType

B, S, D = 4, 4096, 1024
NC = 8               # cores
SC = 512             # s-chunk
NSC = S // SC        # 8 s-chunks
ND = D // 128        # 8 d-chunks
NE = D // 128        # 8 e-chunks (full feature range per core)
INV_SQRT_D = 1.0 / np.sqrt(np.float32(D))
EPS = 1e-5

_prog_cache = {}


def _build_program(use_cbias=False):
    key = ("bpc", use_cbias)
    if key in _prog_cache:
        return _prog_cache[key]

    # All activation funcs used below (Copy/Square/Exp/Ln/Identity) live in
    # the natural_log_exp_and_others table set; restricting the table list
    # lets the act-table pass hoist a single load to program entry instead
    # of thrashing between per-function first-match sets.
    _orig_tables = bacc.get_activation_tables
    _tgt_set = "natural_log_exp_and_others"
    _my_funcs = {AF.Exp, AF.Ln, AF.Copy, AF.Square, AF.Identity}

    def _one_table(arch):
        tabs = _orig_tables(arch)
        if _tgt_set not in tabs or not _my_funcs <= set(tabs[_tgt_set]):
            return tabs
        # act_func_set_id is positional: keep every entry in order, but
        # make _tgt_set the only set claiming the functions we use.
        return {k: (v if k == _tgt_set else set(v) - _my_funcs)
                for k, v in tabs.items()}

    bacc.get_activation_tables = _one_table

    nc = bacc.Bacc("TRN2", num_devices=NC)
    f32, bf16, f8 = dt.float32, dt.bfloat16, dt.float8e4

    # ---- external I/O ----
    hb = nc.dram_tensor("hb", [D, S], bf16, kind="ExternalInput")
    wqT = nc.dram_tensor("wqT", [D, D], bf16, kind="ExternalInput")
    wkT = nc.dram_tensor("wkT", [D, D], bf16, kind="ExternalInput")
    svq_in = nc.dram_tensor("svq", [128, ND], bf16, kind="ExternalInput")
    ccq_in = nc.dram_tensor("ccq", [128, NE], f32, kind="ExternalInput")
    cck_in = nc.dram_tensor("cck", [128, NE], f32, kind="ExternalInput")
    wkp_in = nc.dram_tensor("wkp", [128, NE], bf16, kind="ExternalInput")
    mrow1_in = nc.dram_tensor("mrow1", [1, S], f32, kind="ExternalInput")
    mrow2_in = nc.dram_tensor("mrow2", [1, S], f32, kind="ExternalInput")

    outT = nc.dram_tensor("outT", [D, S], bf16, kind="ExternalOutput")

    with tile.TileContext(nc) as tc:
        with (
            tc.tile_pool(name="const", bufs=1) as cpool,
            tc.tile_pool(name="persist", bufs=1) as ppool,
            tc.tile_pool(name="rows", bufs=1) as rows,
            tc.tile_pool(name="bcast", bufs=2) as bc,
            tc.tile_pool(name="ht", bufs=2) as wht,
            tc.tile_pool(name="sq", bufs=1) as wsq,
            tc.tile_pool(name="qa", bufs=3) as wqa,
            tc.tile_pool(name="ka", bufs=2) as wqk,
            tc.tile_pool(name="scr", bufs=2) as wscr,
            tc.tile_pool(name="nsc", bufs=2) as wnsc,
            tc.tile_pool(name="mk", bufs=2) as wmk,
            tc.tile_pool(name="o", bufs=1) as wo,
            tc.tile_pool(name="psA", bufs=2, space="PSUM") as psA,
            tc.tile_pool(name="psR", bufs=1, space="PSUM") as psR,
            tc.tile_pool(name="psL2", bufs=2, space="PSUM") as psL2,
        ):
            # ---- resident constants ----
            wq_t = cpool.tile([128, ND, D], bf16, tag="wq")
            wk_t = cpool.tile([128, ND, D], bf16, tag="wk")
            nc.sync.dma_start(
                out=wq_t[:], in_=wqT.rearrange("(a p) e -> p a e", p=128))
            nc.sync.dma_start(
                out=wk_t[:], in_=wkT.rearrange("(a p) e -> p a e", p=128))

            svq_t = cpool.tile([128, ND], bf16, tag="svq")
            nc.sync.dma_start(out=svq_t[:], in_=svq_in[:])

            ccq_t = cpool.tile([128, NE], f32, tag="ccq")
            cck_t = cpool.tile([128, NE], f32, tag="cck")
            nc.sync.dma_start(out=ccq_t[:], in_=ccq_in[:])
            nc.sync.dma_start(out=cck_t[:], in_=cck_in[:])

            wkp_t = cpool.tile([128, NE], bf16, tag="wkp")
            nc.sync.dma_start(out=wkp_t[:], in_=wkp_in[:])

            ones8 = cpool.tile([128, 2, 16], f8, tag="ones8")
            nc.vector.memset(ones8[:], 1.0)
            eps_t = cpool.tile([1, 1], f32, tag="eps")
            nc.vector.memset(eps_t[:], EPS)

            # ---- persistent carries ----
            carry_q = ppool.tile([128, NE], bf16, tag="carry_q")
            carry_k = ppool.tile([128, NE], bf16, tag="carry_k")
            carry_d = ppool.tile([1, 2], f32, tag="carry_d")
            nc.vector.memset(carry_q[:], 0.0)
            nc.vector.memset(carry_k[:], 0.0)
            nc.vector.memset(carry_d[:], 0.0)

            hh = ND // 2
            fstate = {}
            state = {}

            def front(cc):
                s0 = cc * SC
                ht = wht.tile([128, ND, SC], bf16, tag="ht")
                nc.sync.dma_start(
                    out=ht[:],
                    in_=hb.rearrange("(a p) s -> p a s",
                                     p=128)[:, :, s0:s0 + SC])

                h8 = wsq.tile([128, ND, SC], f8, tag="h8")
                nc.vector.tensor_copy(h8[:], ht[:])
                sq = wsq.tile([128, ND, SC], f8, tag="sq")
                nc.scalar.activation(sq[:], ht[:], AF.Square)

                # stats rows: st/sxx via fp8 DoubleRow, l1p via svq stationary
                st_ps = psR.tile([1, SC], f32, tag="st")
                for d in range(0, ND, 2):
                    nc.tensor.matmul(
                        st_ps[:], ones8[:, :, 0:1], h8[:, d:d + 2, :],
                        start=(d == 0), stop=(d == ND - 2),
                        perf_mode=mybir.MatmulPerfMode.DoubleRow)
                sxx_ps = psR.tile([1, SC], f32, tag="sxx")
                for d in range(0, ND, 2):
                    nc.tensor.matmul(
                        sxx_ps[:], ones8[:, :, 0:1], sq[:, d:d + 2, :],
                        start=(d == 0), stop=(d == ND - 2),
                        perf_mode=mybir.MatmulPerfMode.DoubleRow)
                l1p_ps = psR.tile([1, SC], f32, tag="l1p")
                for d in range(ND):
                    nc.tensor.matmul(l1p_ps[:], svq_t[:, d:d + 1], ht[:, d, :],
                                     start=(d == 0), stop=(d == ND - 1))

                # LN rows
                musq = rows.tile([1, SC], f32, tag="musq")
                nc.scalar.activation(musq[:], st_ps[:], AF.Square,
                                     scale=1.0 / D)
                var = rows.tile([1, SC], f32, tag="var")
                nc.vector.scalar_tensor_tensor(
                    var[:], sxx_ps[:], 1.0 / D, musq[:],
                    OP.mult, OP.subtract)
                lnv = rows.tile([1, SC], f32, tag="lnv")
                nc.scalar.activation(lnv[:], var[:], AF.Ln, bias=eps_t[:])
                rstd = rows.tile([1, SC], f32, tag="rstd")
                nc.scalar.activation(rstd[:], lnv[:], AF.Exp, scale=-0.5)
                rstd_h = rows.tile([1, SC], bf16, tag="rstd_h")
                nc.scalar.activation(rstd_h[:], lnv[:], AF.Exp, scale=-0.5)
                nmur = rows.tile([1, SC], bf16, tag="nmur")
                nc.vector.scalar_tensor_tensor(
                    nmur[:], st_ps[:], -1.0 / D, rstd[:],
                    OP.mult, OP.mult)

                # l1 -> qw
                l1f = rows.tile([1, SC], f32, tag="l1f")
                nc.vector.tensor_mul(l1f[:], l1p_ps[:], rstd[:])
                m1s = rows.tile([1, SC], f32, tag="m1s")
                nc.sync.dma_start(out=m1s[:], in_=mrow1_in[:, s0:s0 + SC])
                l1b = rows.tile([1, SC], f32, tag="l1b")
                nc.vector.tensor_add(l1b[:], l1f[:], m1s[:])
                qw = rows.tile([1, SC], bf16, tag="qw")
                nc.scalar.activation(qw[:], l1b[:], AF.Exp)

                # den1 scan + rden1 = exp(-ln(den1))
                den1 = rows.tile([1, SC], f32, tag="den1")
                init1 = 0.0 if cc == 0 else carry_d[:, 0:1]
                nc.vector.tensor_tensor_scan(
                    den1[:], qw[:], qw[:], init1, OP.add, OP.bypass)
                nc.vector.tensor_copy(carry_d[:, 0:1], den1[:, SC - 1:SC])
                lnd1 = rows.tile([1, SC], f32, tag="lnd1")
                nc.scalar.activation(lnd1[:], den1[:], AF.Ln)
                rden1h = rows.tile([1, SC], bf16, tag="rden1h")
                nc.scalar.activation(rden1h[:], lnd1[:], AF.Exp, scale=-1.0)

                # broadcasts (Pool)
                rstd_b = bc.tile([128, SC], bf16, tag="rstd_b")
                nc.gpsimd.partition_broadcast(rstd_b[:], rstd_h[:])
                nmur_b = bc.tile([128, SC], bf16, tag="nmur_b")
                nc.gpsimd.partition_broadcast(nmur_b[:], nmur[:])
                qb = bc.tile([128, SC], bf16, tag="qb")
                nc.gpsimd.partition_broadcast(qb[:], qw[:])
                db = bc.tile([128, SC], bf16, tag="db")
                nc.gpsimd.partition_broadcast(db[:], rden1h[:])

                # xs = ht*rstd + nmur  (in-place, halves)
                for p0 in range(2):
                    sl = slice(p0 * hh, (p0 + 1) * hh)
                    nc.vector.tensor_mul(
                        ht[:, sl, :], ht[:, sl, :],
                        rstd_b[:].unsqueeze(1).broadcast_to([128, hh, SC]))
                for p0 in range(2):
                    sl = slice(p0 * hh, (p0 + 1) * hh)
                    nc.vector.tensor_add(
                        ht[:, sl, :], ht[:, sl, :],
                        nmur_b[:].unsqueeze(1).broadcast_to([128, hh, SC]))
                fstate[cc] = (ht, qb, db)

            def mid(cc):
                xs, qb, db = fstate.pop(cc)

                # projections
                q_all = wqa.tile([128, NE, SC], bf16, tag="q_all")
                k_all = wqk.tile([128, NE, SC], bf16, tag="k_all")
                for e in range(NE):
                    es = slice(e * 128, (e + 1) * 128)
                    q_ps = psA.tile([128, SC], f32, tag="proj")
                    for d in range(ND):
                        nc.tensor.matmul(
                            q_ps[:], wq_t[:, d, es], xs[:, d, :],
                            start=(d == 0), stop=(d == ND - 1))
                    if use_cbias:
                        nc.scalar.activation(q_all[:, e, :], q_ps[:],
                                             AF.Identity,
                                             bias=ccq_t[:, e:e + 1])
                    else:
                        nc.scalar.copy(q_all[:, e, :], q_ps[:])
                    k_ps = psA.tile([128, SC], f32, tag="proj")
                    for d in range(ND):
                        nc.tensor.matmul(
                            k_ps[:], wk_t[:, d, es], xs[:, d, :],
                            start=(d == 0), stop=(d == ND - 1))
                    if use_cbias:
                        nc.scalar.activation(k_all[:, e, :], k_ps[:],
                                             AF.Identity,
                                             bias=cck_t[:, e:e + 1])
                    else:
                        nc.scalar.copy(k_all[:, e, :], k_ps[:])

                # u1 = qb * q ; n1 scans ; k' = db*k ; mk = n1*k'
                u1 = wscr.tile([128, NE, SC], bf16, tag="u")
                nc.vector.tensor_mul(
                    u1[:], q_all[:],
                    qb[:].unsqueeze(1).broadcast_to([128, NE, SC]))
                n1 = wnsc.tile([128, NE, SC], bf16, tag="n")
                for e in range(NE):
                    init = 0.0 if cc == 0 else carry_q[:, e:e + 1]
                    nc.vector.tensor_tensor_scan(
                        n1[:, e, :], u1[:, e, :], u1[:, e, :], init,
                        OP.add, OP.bypass)
                    nc.vector.tensor_copy(carry_q[:, e:e + 1],
                                          n1[:, e, SC - 1:SC])
                nc.vector.tensor_mul(
                    k_all[:], k_all[:],
                    db[:].unsqueeze(1).broadcast_to([128, NE, SC]))
                mk = wmk.tile([128, NE, SC], bf16, tag="mk")
                nc.vector.tensor_mul(mk[:], n1[:], k_all[:])

                state[cc] = (q_all, mk, cc * SC)

            def pool2(cc):
                q_all, mk, s0 = state.pop(cc)
                # l2 row
                l2_ps = psL2.tile([1, SC], f32, tag="l2")
                for e in range(NE):
                    nc.tensor.matmul(l2_ps[:], wkp_t[:, e:e + 1],
                                     mk[:, e, :],
                                     start=(e == 0), stop=(e == NE - 1))
                m2s = rows.tile([1, SC], f32, tag="m2s")
                nc.sync.dma_start(out=m2s[:], in_=mrow2_in[:, s0:s0 + SC])
                lg2 = rows.tile([1, SC], f32, tag="lg2")
                nc.vector.tensor_add(lg2[:], l2_ps[:], m2s[:])
                kw = rows.tile([1, SC], bf16, tag="kw")
                nc.scalar.activation(kw[:], lg2[:], AF.Exp)

                den2 = rows.tile([1, SC], f32, tag="den2")
                init2 = 0.0 if s0 == 0 else carry_d[:, 1:2]
                nc.vector.tensor_tensor_scan(
                    den2[:], kw[:], kw[:], init2, OP.add, OP.bypass)
                nc.vector.tensor_copy(carry_d[:, 1:2], den2[:, SC - 1:SC])
                lnd2 = rows.tile([1, SC], f32, tag="lnd2")
                nc.scalar.activation(lnd2[:], den2[:], AF.Ln)
                rden2h = rows.tile([1, SC], bf16, tag="rden2h")
                nc.scalar.activation(rden2h[:], lnd2[:], AF.Exp, scale=-1.0)

                kb = bc.tile([128, SC], bf16, tag="kb")
                nc.gpsimd.partition_broadcast(kb[:], kw[:])
                d2b = bc.tile([128, SC], bf16, tag="d2b")
                nc.gpsimd.partition_broadcast(d2b[:], rden2h[:])

                u2 = wscr.tile([128, NE, SC], bf16, tag="u")
                nc.vector.tensor_mul(
                    u2[:], mk[:],
                    kb[:].unsqueeze(1).broadcast_to([128, NE, SC]))
                n2 = wnsc.tile([128, NE, SC], bf16, tag="n")
                for e in range(NE):
                    init = 0.0 if s0 == 0 else carry_k[:, e:e + 1]
                    nc.vector.tensor_tensor_scan(
                        n2[:, e, :], u2[:, e, :], u2[:, e, :], init,
                        OP.add, OP.bypass)
                    nc.vector.tensor_copy(carry_k[:, e:e + 1],
                                          n2[:, e, SC - 1:SC])
                # o = (n2*q) * rden2b — n2*q first so q_all frees early
                o1 = wscr.tile([128, NE, SC], bf16, tag="u")
                nc.vector.tensor_mul(o1[:], n2[:], q_all[:])
                o = wo.tile([128, NE, SC], bf16, tag="o")
                nc.vector.tensor_mul(
                    o[:], o1[:],
                    d2b[:].unsqueeze(1).broadcast_to([128, NE, SC]))
                nc.sync.dma_start(
                    out=outT.rearrange("(a p) s -> p a s",
                                       p=128)[:, :, s0:s0 + SC],
                    in_=o[:])

            for cc in range(NSC + 2):
                if cc < NSC:
                    front(cc)
                if 1 <= cc <= NSC:
                    mid(cc - 1)
                if cc >= 2:
                    pool2(cc - 2)

    try:
        nc.finalize()
    finally:
        bacc.get_activation_tables = _orig_tables
    _prog_cache[key] = nc
    return nc


def _host_prep(hidden_states, attention_mask, Wq, wq_att, Wk, wk_att, ln_g, ln_b):
    """Build the 8 per-core input maps (batch b = core % 4)."""
    f4 = np.float32
    g = np.asarray(ln_g, f4)
    bb = np.asarray(ln_b, f4)
    Wq = np.asarray(Wq, f4)
    Wk = np.asarray(Wk, f4)
    wq_att = np.asarray(wq_att, f4)[:, 0]
    wk_att = np.asarray(wk_att, f4)[:, 0]
    h = np.asarray(hidden_states, f4)
    am = np.asarray(attention_mask, f4)

    Wqp = Wq * g[None, :]           # [e,d]
    Wkp = Wk * g[None, :]
    wqT_full = np.ascontiguousarray(Wqp.T)   # [d,e]
    wkT_full = np.ascontiguousarray(Wkp.T)
    cq = Wq @ bb                    # [e] (zero when ln_b == 0)
    ck = Wk @ bb

    vq = Wq.T @ wq_att              # [d]
    vqp = (g * vq) * INV_SQRT_D     # [d]
    cvq = float(bb @ vq) * INV_SQRT_D
    colsvq = float(vqp.sum())
    wkp_full = (wk_att * INV_SQRT_D).astype(f4)

    maskb = (1.0 - am) * -10000.0   # [B,S]

    def bf(a):
        return np.ascontiguousarray(
            np.asarray(a, f4).astype(ml_dtypes.bfloat16))

    # stationary [svq | ones]: svq folds the l1 mean subtraction
    svq = np.ascontiguousarray((vqp - colsvq / D).reshape(ND, 128).T)  # [128, ND]

    ccq = np.ascontiguousarray(cq.reshape(NE, 128).T)       # [128, NE]
    cck = np.ascontiguousarray(ck.reshape(NE, 128).T)
    wkp = bf(wkp_full.reshape(NE, 128).T)                   # [128, NE]

    in_maps = []
    for core in range(NC):
        b = core % B
        in_maps.append({
            "hb": bf(h[b].T),
            "wqT": bf(wqT_full),
            "wkT": bf(wkT_full),
            "svq": bf(svq),
            "ccq": ccq.astype(f4),
            "cck": cck.astype(f4),
            "wkp": wkp,
            "mrow1": np.ascontiguousarray((maskb[b] + cvq).reshape(1, S)),
            "mrow2": np.ascontiguousarray(maskb[b].reshape(1, S)),
        })
    return in_maps, bool(np.any(cq != 0.0) or np.any(ck != 0.0))


def kernel(**inputs):
    import time as _time
    in_maps, use_cbias = _host_prep(**inputs)
    nc = _build_program(use_cbias=use_cbias)
    res = None
    last = None
    for _attempt in range(3):
        try:
            res = run_bass_kernel_spmd(nc, in_maps, core_ids=list(range(NC)))
            break
        except Exception as e:  # transient first-exec device faults self-heal
            last = e
            _time.sleep(3)
    if res is None:
        raise last
    out = np.empty((B, S, D), np.float32)
    for b in range(B):
        out[b] = res.results[b]["outT"].astype(np.float32).T
    return out
